# revision 1
# baseline (speedup 1.0000x reference)
"""Single-head causal attention (B=4, S=4096, D=128, fp32) on 8 Trainium2 cores.

Sharding: core c = (batch b = c//2, fold f = c%2). Each core processes ALL
queries of its batch but only the keys in 128-row chunks at global offsets
256*k + 128*f (k = 0..15). This interleaving makes the causal schedule
identical on every core (SPMD requires one program), while host-side input
prep (pre-transposed x, compacted kv rows, mask tiles passed as data) absorbs
all per-core differences into the input data.

Device outputs per core: unnormalized out^T partial [128, 4096] and softmax
denominator partial [1, 4096]. Host combines the two folds per batch:
out[b] = ((outT0 + outT1) / (den0 + den1)).T
"""

import numpy as np
from contextlib import ExitStack

import concourse.bacc as bacc
import concourse.tile as tile
import concourse.mybir as mybir
from concourse.bass_utils import run_bass_kernel_spmd

B, S, D = 4, 4096, 128
NCORES = 8
QB = 512          # query block (matmul moving dim)
CK = 128          # key chunk (matmul stationary dim)
NQB = S // QB     # 8 query blocks
NCK = 16          # key chunks per core (S/2/CK)
SCALE = float(1.0 / np.sqrt(D))
WARMUP_MMS = 11   # dummy matmuls to ramp the PE clock during the DMA head

FP32 = mybir.dt.float32
FP32R = mybir.dt.float32r

# fp32r = TF32-like fast matmul mode (1 cyc/row vs 4 for fp32; inputs
# effectively rounded to 11 mantissa bits). Measured end-to-end absmax vs
# reference: all-fp32r ~1.5e-3, all-fp32 ~7e-6. Walrus requires every
# producer of matmul-consumed data to emit float32r, so the SBUF tiles
# themselves carry the dtype.
USE_FP32R = True
DTM = FP32R if USE_FP32R else FP32

_CACHE = {}


def _bc(ap):
    return ap.bitcast(DTM) if DTM != FP32 else ap


def _build():
    nc = bacc.Bacc("TRN2", target_bir_lowering=False, debug=False)

    xqT = nc.dram_tensor("xqT", [D, S], FP32, kind="ExternalInput").ap()
    xkvT = nc.dram_tensor("xkvT", [D, S // 2], FP32, kind="ExternalInput").ap()
    wpack = nc.dram_tensor("wpack", [D, 3 * D], FP32, kind="ExternalInput").ap()
    # masks + ones column: maskA | maskB | 1
    mpack = nc.dram_tensor("mpack", [CK, 2 * QB + 1], FP32,
                           kind="ExternalInput").ap()

    outT = nc.dram_tensor("outT", [D, S], FP32, kind="ExternalOutput").ap()
    den = nc.dram_tensor("den", [1, S], FP32, kind="ExternalOutput").ap()

    with tile.TileContext(nc) as tc, ExitStack() as ctx:
        consts = ctx.enter_context(tc.tile_pool(name="consts", bufs=1))
        stage = ctx.enter_context(tc.tile_pool(name="stage", bufs=2))
        ptp = ctx.enter_context(tc.tile_pool(name="ptp", bufs=5))
        ps_s = ctx.enter_context(tc.tile_pool(name="ps_s", bufs=3, space="PSUM"))
        ps_o = ctx.enter_context(tc.tile_pool(name="ps_o", bufs=1, space="PSUM"))
        ps_d = ctx.enter_context(tc.tile_pool(name="ps_d", bufs=1, space="PSUM"))

        # ---- PE warm-up: dummy matmuls on zeroed scratch, no load deps ----
        t_z = consts.tile([D, QB], DTM, tag="z")
        nc.vector.memset(t_z.bitcast(FP32)[:], 0.0)

        def dummy_mm():
            pz = ps_s.tile([CK, 2 * QB], FP32, tag="s", name="pz")
            nc.tensor.matmul(pz[:, 0:QB], t_z[:, 0:CK], t_z[:],
                             start=True, stop=True)

        for _ in range(WARMUP_MMS):
            dummy_mm()

        # ---- loads: one queue, ordered by when compute consumes the data ----
        t_wp = consts.tile([D, 3 * D], DTM, tag="wp")
        t_xkv = consts.tile([D, S // 2], DTM, tag="xkv")
        t_xq = consts.tile([D, S], DTM, tag="xq")
        t_mp = consts.tile([CK, 2 * QB + 1], DTM, tag="mp")

        def ld(dst, src_ap):
            nc.sync.dma_start(dst, src_ap)

        ld(t_wp[:], _bc(wpack)[:])
        ld(t_xkv[:, 0:512], _bc(xkvT)[:, 0:512])
        ld(t_xq[:, 512:1024], _bc(xqT)[:, 512:1024])
        ld(t_mp[:], _bc(mpack)[:])
        ld(t_xkv[:, 512:1024], _bc(xkvT)[:, 512:1024])
        ld(t_xq[:, 1024:1536], _bc(xqT)[:, 1024:1536])
        ld(t_xkv[:, 1024:2048], _bc(xkvT)[:, 1024:2048])
        ld(t_xq[:, 1536:2048], _bc(xqT)[:, 1536:2048])
        ld(t_xq[:, 0:512], _bc(xqT)[:, 0:512])
        ld(t_xq[:, 2048:3072], _bc(xqT)[:, 2048:3072])
        ld(t_xq[:, 3072:4096], _bc(xqT)[:, 3072:4096])

        t_wq = t_wp[:, 0:D]
        t_wk = t_wp[:, D:2 * D]
        t_wv = t_wp[:, 2 * D:3 * D]
        t_mA = t_mp[:, 0:QB]
        t_mB = t_mp[:, QB:2 * QB]
        t_ones = t_mp[:, 2 * QB:2 * QB + 1]

        # ---- projections (emitted interleaved with attention blocks, in the
        # order the DMA queue delivers their inputs) ----
        t_KT = consts.tile([D, S // 2], DTM, tag="KT")
        t_QT = consts.tile([D, S], DTM, tag="QT")
        t_V = consts.tile([CK, NCK * D], DTM, tag="V")
        t_den = consts.tile([1, S], FP32, tag="den")

        def proj_kt(t, halves=False):   # 512 compacted keys = chunks 4t..4t+3
            pk = ps_s.tile([D, 2 * QB], FP32, tag="s")
            if halves:
                for h in range(2):
                    sl = slice(t * QB + h * 256, t * QB + (h + 1) * 256)
                    nc.tensor.matmul(pk[:, h * 256:(h + 1) * 256], t_wk,
                                     t_xkv[:, sl], start=True, stop=True)
            else:
                nc.tensor.matmul(pk[:, 0:QB], t_wk,
                                 t_xkv[:, t * QB:(t + 1) * QB],
                                 start=True, stop=True)
            nc.vector.tensor_copy(t_KT[:, t * QB:(t + 1) * QB], pk[:, 0:QB])

        def proj_qt(t):   # query block t
            pq = ps_s.tile([D, 2 * QB], FP32, tag="s")
            nc.tensor.matmul(pq[:, 0:QB], t_wq, t_xq[:, t * QB:(t + 1) * QB],
                             start=True, stop=True)
            nc.scalar.copy(t_QT[:, t * QB:(t + 1) * QB], pq[:, 0:QB])

        def proj_v(t):    # chunks 4t..4t+3
            pv = ps_s.tile([D, 2 * QB], FP32, tag="s")
            for h in range(4):
                k = 4 * t + h
                nc.tensor.matmul(pv[:, h * D:(h + 1) * D],
                                 t_xkv[:, k * CK:(k + 1) * CK], t_wv,
                                 start=True, stop=True)
            nc.vector.tensor_copy(t_V[:, t * QB:(t + 1) * QB], pv[:, 0:QB])

        # ---- attention: flat unit stream, software-pipelined so each
        # unit's PV/ones matmuls are emitted after the NEXT unit's S^T
        # matmuls (PE is in-order; this hides the exp latency). ----
        JORDER = [1, 2, 3, 0, 4, 5, 6, 7]
        # last block runs its diagonal group first so the final unit has no
        # mask-mul chain in the tail
        def groups(j):
            if j == JORDER[-1]:
                return [j] + list(range(j))
            return list(range(j + 1))
        units = [(j, g) for j in JORDER for g in groups(j)]
        pt_of = {}
        po_of = {}
        pd_of = {}

        # projections interleaved at block starts, matching DMA arrival
        projs_at = {
            1: [lambda: proj_kt(0, halves=True), lambda: proj_v(0),
                lambda: proj_qt(1)],
            2: [lambda: proj_kt(1), lambda: proj_v(1), lambda: proj_qt(2)],
            3: [lambda: proj_kt(2), lambda: proj_kt(3), lambda: proj_v(2),
                lambda: proj_v(3), lambda: proj_qt(3)],
            0: [lambda: proj_qt(0)],
            4: [lambda: proj_qt(4)],
            5: [lambda: proj_qt(5)],
            6: [lambda: proj_qt(6)],
            7: [lambda: proj_qt(7)],
        }

        def emit_S(u):
            j, g = units[u]
            qs = slice(j * QB, (j + 1) * QB)
            ka, kb = 2 * g, 2 * g + 1
            pst = ps_s.tile([CK, 2 * QB], FP32, tag="s")
            nc.tensor.matmul(pst[:, 0:QB],
                             t_KT[:, ka * CK:(ka + 1) * CK], t_QT[:, qs],
                             start=True, stop=True)
            nc.tensor.matmul(pst[:, QB:2 * QB],
                             t_KT[:, kb * CK:(kb + 1) * CK], t_QT[:, qs],
                             start=True, stop=True)
            pt = ptp.tile([CK, 2 * QB], DTM, tag="pt")
            nc.scalar.activation(pt[:], pst[:],
                                 mybir.ActivationFunctionType.Exp,
                                 scale=SCALE)
            if g == j:            # the two diagonal chunks
                nc.vector.tensor_mul(pt[:, 0:QB], pt[:, 0:QB], t_mA)
                nc.vector.tensor_mul(pt[:, QB:2 * QB], pt[:, QB:2 * QB], t_mB)
            pt_of[u] = pt

        def emit_PV(u):
            j, g = units[u]
            qs = slice(j * QB, (j + 1) * QB)
            ka, kb = 2 * g, 2 * g + 1
            if j == JORDER[-1]:
                first = (g == j)
                last = (g == j - 1)
            else:
                first = (g == 0)
                last = (g == j)
            if first:
                po_of[j] = ps_o.tile([D, QB], FP32, tag="o", name="po")
                pd_of[j] = ps_d.tile([1, QB], FP32, tag="d", name="pd")
            po, pd_ = po_of[j], pd_of[j]
            pt = pt_of.pop(u)
            nc.tensor.matmul(po[:], t_V[:, ka * D:(ka + 1) * D], pt[:, 0:QB],
                             start=first, stop=False)
            nc.tensor.matmul(po[:], t_V[:, kb * D:(kb + 1) * D],
                             pt[:, QB:2 * QB],
                             start=False, stop=last)
            if last:              # drain out^T as soon as its group closes
                so = stage.tile([D, QB], FP32, tag="so")
                nc.vector.tensor_copy(so[:], po[:])
                nc.sync.dma_start(outT[:, qs], so[:])
            nc.tensor.matmul(pd_[:], t_ones, pt[:, 0:QB],
                             start=first, stop=False)
            nc.tensor.matmul(pd_[:], t_ones, pt[:, QB:2 * QB],
                             start=False, stop=last)
            if last:
                nc.vector.tensor_copy(t_den[0:1, qs], pd_[:])

        LOOKAHEAD = 2
        started = set()
        for u in range(len(units)):
            j, g = units[u]
            if j not in started:
                started.add(j)
                for p in projs_at.get(j, []):
                    p()
            emit_S(u)
            if u >= LOOKAHEAD:
                emit_PV(u - LOOKAHEAD)
        for u in range(len(units) - LOOKAHEAD, len(units)):
            emit_PV(u)
        nc.scalar.dma_start(den[:], t_den[:])

    nc.compile()
    return nc


def get_nc():
    if "nc" not in _CACHE:
        _CACHE["nc"] = _build()
    return _CACHE["nc"]


def make_in_maps(x, Wq, Wk, Wv):
    x = np.ascontiguousarray(np.asarray(x, dtype=np.float32))
    wqT = np.ascontiguousarray(np.asarray(Wq, dtype=np.float32).T)
    wkT = np.ascontiguousarray(np.asarray(Wk, dtype=np.float32).T)
    wvT = np.ascontiguousarray(np.asarray(Wv, dtype=np.float32).T)
    wpack = np.ascontiguousarray(np.concatenate([wqT, wkT, wvT], axis=1))

    kk = np.arange(CK)[:, None]
    qq = np.arange(QB)[None, :]
    in_maps = []
    for c in range(NCORES):
        b, f = c // 2, c % 2
        xb = x[b]                       # [S, D]
        xqT = np.ascontiguousarray(xb.T)
        rows = (np.arange(S // 2) // CK) * 256 + CK * f + (np.arange(S // 2) % CK)
        xkvT = np.ascontiguousarray(xb[rows].T)
        maskA = (qq - kk >= CK * f).astype(np.float32)
        maskB = (qq - kk >= 256 + CK * f).astype(np.float32)
        mpack = np.concatenate(
            [maskA, maskB, np.ones((CK, 1), np.float32)], axis=1)
        in_maps.append({
            "xqT": xqT, "xkvT": xkvT,
            "wpack": wpack,
            "mpack": np.ascontiguousarray(mpack),
        })
    return in_maps


def combine(results):
    out = np.empty((B, S, D), np.float32)
    for b in range(B):
        o0 = results[2 * b]["outT"].astype(np.float64)
        o1 = results[2 * b + 1]["outT"].astype(np.float64)
        d0 = results[2 * b]["den"].astype(np.float64)
        d1 = results[2 * b + 1]["den"].astype(np.float64)
        out[b] = (((o0 + o1) / (d0 + d1)).T).astype(np.float32)
    return out


def kernel(x, Wq, Wk, Wv):
    nc = get_nc()
    in_maps = make_in_maps(x, Wq, Wk, Wv)
    res = run_bass_kernel_spmd(nc, in_maps, core_ids=list(range(NCORES)))
    return combine(res.results)


if __name__ == "__main__":
    import reference
    inputs = reference.setup_inputs()
    expected = np.asarray(reference.reference(**inputs))
    actual = kernel(**{k: np.asarray(v) for k, v in inputs.items()})
    err = np.abs(actual - expected).max()
    print("absmax err:", err, " scale:", np.abs(expected).max())

